# revision 30
# baseline (speedup 1.0000x reference)
"""Trainium2 Bass kernel for a 3-layer GraphSAGE GNN (EnhancedSAGE).

Reference computation (see problem statement):
    h  = relu(BN(sage_conv(x, A, Wl0, bl0, Wr0), g0, b0))
    h  = relu(BN(sage_conv(h, A, Wl1, bl1, Wr1), g1, b1))
    out = log_softmax(sage_conv(h, A, Wlo, blo, Wro))
with sage_conv(x) = (mean over in-neighbors of x_src) @ Wl + bl + x @ Wr and
BN = batchnorm over the node dimension.

Distribution / algorithm (8 NeuronCores, graph parallel):
  * Nodes are permuted by in-degree rank and padded to 50176 positions =
    392 blocks x 128 lanes; block b (rank-contiguous, so near-uniform degree)
    goes to core b%8, slot b//8.  Padded table row = core*6272 + slot*128
    + lane, which is exactly the AllGather(core-major) layout.
  * Neighbor aggregation: for destination slot s and neighbor ordinal k, one
    indirect-gather DMA with CCE accumulate adds table[idx[:, s, k]] into the
    slot's SBUF accumulator row-per-destination (first ordinal plain write).
    Degree sorting makes the per-slot neighbor count (max degree in block)
    tight, so descriptor count tracks edge count.  Index padding points at a
    zeroed table row.  x / h tables are replicated in every core's HBM.
  * The summed aggregate is scaled per destination by 1/deg (exact mean),
    transposed on the tensor engine to feature-major, and transformed with
    256-destination-wide float32r matmuls (full PE rate).
  * Feature-major activations let BatchNorm scale/shift/ReLU fuse into one
    scalar-engine activation per block; BN stats use a [128, 2] AllReduce;
    layer outputs are transposed back and AllGathered node-major for the
    next layer's gather table.
"""

import numpy as np

import concourse.bass as bass
import concourse.bacc as bacc
import concourse.tile as tile
import concourse.mybir as mybir
from concourse import bass_utils

P = 128
NCORES = 8
SLOTS = 49                 # 128-node blocks per core
SS = (SLOTS + 1) // 2      # 256-node superslots per core (last is 128 wide)
N, E, F, H, C = 50000, 600000, 128, 128, 47
CP = 48                    # class dim padded for f32r matmul (N must be even)
RPC = SLOTS * P            # rows per core (6272)
NPAD = NCORES * RPC        # padded node count (50176)
TROWS = NPAD               # table rows (pad index -> a dead, masked-to-zero row)
EPS = 1e-5

f32 = mybir.dt.float32
f32r = mybir.dt.float32r
i32 = mybir.dt.int32
AF = mybir.ActivationFunctionType
OP = mybir.AluOpType
AX = mybir.AxisListType
RG = [list(range(NCORES))]

LAST_RESULT = None  # test harness peeks at this for profiling info


def _ss_width(ss):
    return 256 if 2 * ss + 1 < SLOTS else 128


# --------------------------------------------------------------------------
# Host-side preprocessing
# --------------------------------------------------------------------------

def _preprocess(edge_index):
    src = np.asarray(edge_index[0], np.int64)
    dst = np.asarray(edge_index[1], np.int64)
    cnt = np.bincount(dst, minlength=N)

    # degree-ascending node permutation; node v sits at padded position pos[v]
    rank = np.argsort(cnt, kind="stable")        # node ids by degree rank
    blkof = np.arange(N) // P                    # block of each rank
    b = blkof
    posr = (b % NCORES) * RPC + (b // NCORES) * P + np.arange(N) % P
    pos = np.empty(N, np.int64)
    pos[rank] = posr                             # padded row of node v

    # per-destination-slot neighbor lists
    pdst = pos[dst]                              # padded dst position
    core = pdst // RPC
    slot = (pdst % RPC) // P
    lanep = pdst % P
    gslot = core * SLOTS + slot                  # 0..391

    order = np.argsort(gslot * NPAD + lanep, kind="stable")
    src_o = pos[src[order]]
    gs_o = gslot[order]
    lane_o = lanep[order]

    deg = np.bincount(pdst, minlength=NPAD)      # per padded position
    degl = deg.reshape(NCORES, SLOTS, P)
    Tpc = degl.max(axis=2)                       # [cores, slots] max degree
    T = np.maximum(1, Tpc.max(axis=0)).astype(np.int64)   # uniform per slot
    toff = np.zeros(SLOTS + 1, np.int64)
    np.cumsum(T, out=toff[1:])
    t_total = int(T.sum())

    idx = np.full((NCORES, P, t_total), NPAD - 1, np.int32)  # pad -> dead row
    # scatter each edge to its (core, slot, lane, ordinal) cell
    ecnt = np.zeros(NPAD, np.int64)
    # ordinal per edge: position within its destination's list
    # (order is grouped by (gslot, lane); use searchsorted-free cumcount)
    grp = gs_o * P + lane_o
    # cumulative count within equal grp values (grp is sorted)
    first = np.r_[True, grp[1:] != grp[:-1]]
    gstart = np.flatnonzero(first)
    glen = np.diff(np.r_[gstart, len(grp)])
    ordinal = np.arange(len(grp)) - np.repeat(gstart, glen)

    ecore = gs_o // SLOTS
    eslot = gs_o % SLOTS
    cols = toff[eslot] + ordinal
    idx[ecore, lane_o, cols] = src_o.astype(np.int32)

    winv = np.zeros((NCORES, P, SLOTS), np.float32)
    cntp = np.maximum(deg, 1).astype(np.float32)
    winv[:, :, :] = (1.0 / cntp).reshape(NCORES, SLOTS, P).transpose(0, 2, 1)

    # mask for the final slot (only blocks 390/391 contain pad positions)
    mb = np.zeros((NCORES, P, 256), np.float32)
    nvalid = np.zeros(NPAD, np.float32)
    nvalid[posr] = 1.0
    for c in range(NCORES):
        s = SLOTS - 1
        rows = nvalid[c * RPC + s * P : c * RPC + (s + 1) * P]
        mb[c, :, :P] = rows[None, :]
    return T, t_total, idx, winv, mb, pos


# --------------------------------------------------------------------------
# Device program
# --------------------------------------------------------------------------

def _build_program(T, t_total):
    nc = bacc.Bacc(
        "TRN2", target_bir_lowering=False, debug=False, num_devices=NCORES
    )

    din = {}
    for name, shape, dt in [
        ("x_rep", [TROWS, F], f32r),
        ("xownT", [P, RPC], f32r),
        ("idx", [P, t_total], i32),
        ("winv", [P, SLOTS], f32),
        ("ident", [P, P], f32r),
        ("identf", [P, P], f32),
        ("mb", [P, 256], f32),
        ("Wl0", [F, H], f32r), ("Wr0", [F, H], f32r), ("bl0", [H, 1], f32),
        ("g0", [H, 1], f32), ("b0", [H, 1], f32),
        ("Wl1", [H, H], f32r), ("Wr1", [H, H], f32r), ("bl1", [H, 1], f32),
        ("g1", [H, 1], f32), ("b1", [H, 1], f32),
        ("Wlo", [H, CP], f32r), ("Wro", [H, CP], f32r),
        ("blo_mat", [P, CP], f32),
    ]:
        din[name] = nc.dram_tensor(name, shape, dt, kind="ExternalInput").ap()
    out_d = nc.dram_tensor("out_shard", [RPC, C], f32, kind="ExternalOutput").ap()

    toff = np.zeros(SLOTS + 1, np.int64)
    np.cumsum(T, out=toff[1:])
    maxT = int(T.max())

    with tile.TileContext(nc) as tc:
        with (
            tc.tile_pool(name="const", bufs=1) as const,
            tc.tile_pool(name="work", bufs=3) as work,
            tc.tile_pool(name="vec", bufs=1) as vec,
            tc.tile_pool(name="psB", bufs=2, space="PSUM") as psB,
            tc.tile_pool(name="psT", bufs=2, space="PSUM") as psT,
            tc.tile_pool(name="dram", bufs=1, space="DRAM") as dram,
        ):
            def load(name, dt=f32):
                t = const.tile(list(din[name].shape), dt, name=name + "_sb")
                nc.sync.dma_start(t[:], din[name][:])
                return t

            idx_sb = load("idx", i32)
            winv_sb = load("winv")
            mb_sb = load("mb")
            xownT_sb = load("xownT", f32r)
            Wl = [load("Wl0", f32r), load("Wl1", f32r), load("Wlo", f32r)]
            Wr = [load("Wr0", f32r), load("Wr1", f32r), load("Wro", f32r)]
            bl = [load("bl0"), load("bl1")]
            gam = [load("g0"), load("g1")]
            bet = [load("b0"), load("b1")]
            blo_mat_sb = load("blo_mat")
            ident = load("ident", f32r)
            identf = load("identf")

            hpre = const.tile([P, RPC], f32, name="hpre")
            agg_all = const.tile([P, RPC], f32, name="agg_all")
            hT = [
                const.tile([P, RPC], f32r, name="hT0"),
                const.tile([P, RPC], f32r, name="hT1"),
            ]

            hf = [
                dram.tile([TROWS, F], f32r, name="hf0", addr_space="Shared"),
                dram.tile([TROWS, F], f32r, name="hf1", addr_space="Shared"),
            ]
            ag_in = [
                dram.tile([RPC, F], f32r, name="ag_in0"),
                dram.tile([RPC, F], f32r, name="ag_in1"),
            ]


            # ---- one SAGE layer -------------------------------------------
            def layer(li, table_ap, xown, Wl_sb, Wr_sb):
                is_out = li == 2
                if not is_out:
                    sumc = vec.tile([P, SS], f32, name=f"sumc{li}")
                    ssqc = vec.tile([P, SS], f32, name=f"ssqc{li}")

                # neighbor accumulation, neighbor-ordinal-major so that
                # consecutive DMAs hit different slots (no WAW stalls)
                for k in range(maxT):
                    for s in range(SLOTS):
                        if k >= int(T[s]):
                            continue
                        sl = slice(s * P, (s + 1) * P)
                        nc.gpsimd.indirect_dma_start(
                            out=agg_all[:, sl],
                            out_offset=None,
                            in_=table_ap,
                            in_offset=bass.IndirectOffsetOnAxis(
                                ap=idx_sb[:, int(toff[s]) + k :
                                          int(toff[s]) + k + 1],
                                axis=0,
                            ),
                            compute_op=(OP.bypass if k == 0 else OP.add),
                        )

                for ss_i in range(SS):
                    wd = _ss_width(ss_i)
                    agg_sb = work.tile([P, 256], f32r, name="agg_sb")
                    for d in range(wd // P):
                        s = 2 * ss_i + d
                        sl = slice(s * P, (s + 1) * P)
                        # exact mean: per-destination 1/deg, then transpose
                        nc.vector.tensor_scalar_mul(
                            agg_all[:, sl], agg_all[:, sl],
                            winv_sb[:, s : s + 1],
                        )
                        trpa = psT.tile([P, P], f32, name="trpa", tag="trp")
                        nc.tensor.transpose(trpa[:], agg_all[:, sl], identf[:])
                        nc.vector.tensor_copy(
                            agg_sb[:, d * P : (d + 1) * P], trpa[:]
                        )
                    base = 2 * ss_i * P
                    if not is_out:
                        hp = psB.tile([P, 256], f32, name="hp")
                        nc.tensor.matmul(
                            hp[:, :wd], lhsT=Wl_sb[:],
                            rhs=agg_sb[:, :wd],
                            start=True, stop=False,
                        )
                        nc.tensor.matmul(
                            hp[:, :wd], lhsT=Wr_sb[:],
                            rhs=xown[:, base : base + wd],
                            start=False, stop=True,
                        )
                        hs = hpre[:, base : base + wd]
                        sq = work.tile([P, 256], f32, name="sq")
                        if ss_i == SS - 1:
                            nc.scalar.activation(
                                hs, hp[:, :wd], AF.Identity, bias=bl[li][:, :1]
                            )
                            nc.vector.tensor_tensor(
                                out=hs, in0=hs, in1=mb_sb[:, :wd], op=OP.mult
                            )
                            nc.vector.reduce_sum(
                                sumc[:, ss_i : ss_i + 1], hs, axis=AX.X
                            )
                            nc.scalar.activation(
                                sq[:, :wd], hs, AF.Square,
                                accum_out=ssqc[:, ss_i : ss_i + 1],
                            )
                        else:
                            nc.scalar.activation(
                                hs, hp[:, :wd], AF.Identity, bias=bl[li][:, :1],
                                accum_out=sumc[:, ss_i : ss_i + 1],
                            )
                            nc.scalar.activation(
                                sq[:, :wd], hs, AF.Square,
                                accum_out=ssqc[:, ss_i : ss_i + 1],
                            )
                    else:
                        for d in range(wd // P):
                            sl = slice(base + d * P, base + (d + 1) * P)
                            op_ps = psT.tile([P, CP], f32, name="op_ps")
                            nc.tensor.matmul(
                                op_ps[:], lhsT=agg_sb[:, d * P : (d + 1) * P],
                                rhs=Wl_sb[:], start=True, stop=False,
                            )
                            nc.tensor.matmul(
                                op_ps[:], lhsT=xown[:, sl], rhs=Wr_sb[:],
                                start=False, stop=True,
                            )
                            ob = work.tile([P, CP], f32, name="ob")
                            nc.vector.tensor_tensor(
                                out=ob[:], in0=op_ps[:], in1=blo_mat_sb[:],
                                op=OP.add,
                            )
                            mx = work.tile([P, 1], f32, name="mx")
                            nc.vector.reduce_max(mx[:], ob[:], axis=AX.X)
                            mxn = work.tile([P, 1], f32, name="mxn")
                            nc.vector.tensor_scalar_mul(mxn[:], mx[:], -1.0)
                            ex = work.tile([P, CP], f32, name="ex")
                            se = work.tile([P, 1], f32, name="se")
                            nc.scalar.activation(
                                ex[:], ob[:], AF.Exp, bias=mxn[:, :1],
                                accum_out=se[:],
                            )
                            lse = work.tile([P, 1], f32, name="lse")
                            nc.scalar.activation(lse[:], se[:], AF.Ln)
                            tot = work.tile([P, 1], f32, name="tot")
                            nc.vector.tensor_tensor(
                                out=tot[:], in0=lse[:], in1=mx[:], op=OP.add
                            )
                            res = work.tile([P, CP], f32, name="res")
                            nc.vector.tensor_scalar(
                                out=res[:], in0=ob[:], scalar1=tot[:, :1],
                                scalar2=None, op0=OP.subtract,
                            )
                            nc.sync.dma_start(out_d[sl, :], res[:, :C])

                if is_out:
                    return

                # ---- BN statistics (AllReduce) + scale/shift --------------
                S = vec.tile([P, 1], f32, name=f"S{li}")
                SSq = vec.tile([P, 1], f32, name=f"SSq{li}")
                nc.vector.reduce_sum(S[:], sumc[:], axis=AX.X)
                nc.vector.reduce_sum(SSq[:], ssqc[:], axis=AX.X)
                stat = vec.tile([P, 2], f32, name=f"stat{li}")
                nc.vector.tensor_copy(stat[:, 0:1], S[:])
                nc.vector.tensor_copy(stat[:, 1:2], SSq[:])
                cin = dram.tile([P, 2], f32, name=f"cin{li}")
                cout = dram.tile([P, 2], f32, name=f"cout{li}",
                                 addr_space="Shared")
                nc.sync.dma_start(cin[:], stat[:])
                nc.gpsimd.collective_compute(
                    "AllReduce", OP.add, replica_groups=RG,
                    ins=[cin.opt()], outs=[cout.opt()],
                )
                gst = vec.tile([P, 2], f32, name=f"gst{li}")
                nc.sync.dma_start(gst[:], cout[:])
                mu = vec.tile([P, 1], f32, name=f"mu{li}")
                nc.vector.tensor_scalar_mul(mu[:], gst[:, 0:1], 1.0 / N)
                ex2 = vec.tile([P, 1], f32, name=f"ex2{li}")
                nc.vector.tensor_scalar_mul(ex2[:], gst[:, 1:2], 1.0 / N)
                mu2 = vec.tile([P, 1], f32, name=f"mu2{li}")
                nc.vector.tensor_tensor(out=mu2[:], in0=mu[:], in1=mu[:],
                                        op=OP.mult)
                var = vec.tile([P, 1], f32, name=f"var{li}")
                nc.vector.tensor_tensor(out=var[:], in0=ex2[:], in1=mu2[:],
                                        op=OP.subtract)
                sd = vec.tile([P, 1], f32, name=f"sd{li}")
                epsv = vec.tile([P, 1], f32, name=f"epsv{li}")
                nc.vector.memset(epsv[:], EPS)
                nc.scalar.activation(sd[:], var[:], AF.Sqrt, bias=epsv[:, :1])
                rsd = vec.tile([P, 1], f32, name=f"rsd{li}")
                nc.vector.reciprocal(rsd[:], sd[:])
                scl = vec.tile([P, 1], f32, name=f"scl{li}")
                nc.vector.tensor_tensor(out=scl[:], in0=gam[li][:], in1=rsd[:],
                                        op=OP.mult)
                msc = vec.tile([P, 1], f32, name=f"msc{li}")
                nc.vector.tensor_tensor(out=msc[:], in0=mu[:], in1=scl[:],
                                        op=OP.mult)
                sh = vec.tile([P, 1], f32, name=f"sh{li}")
                nc.vector.tensor_tensor(out=sh[:], in0=bet[li][:], in1=msc[:],
                                        op=OP.subtract)

                # ---- phase B: BN+ReLU, transpose, AllGather ---------------
                for s in range(SLOTS):
                    sl = slice(s * P, (s + 1) * P)
                    nc.scalar.activation(
                        hT[li][:, sl], hpre[:, sl], AF.Relu,
                        bias=sh[:, :1], scale=scl[:, :1],
                    )
                    if s == SLOTS - 1:
                        nc.vector.tensor_tensor(
                            out=hT[li][:, sl], in0=hT[li][:, sl],
                            in1=mb_sb[:, :P].bitcast(f32r), op=OP.mult,
                        )
                    trp = psT.tile([P, P], f32r, name="trp")
                    nc.tensor.transpose(trp[:], hT[li][:, sl], ident[:])
                    hnode = work.tile([P, P], f32r, name="hnode")
                    nc.vector.tensor_copy(hnode[:], trp[:])
                    nc.sync.dma_start(ag_in[li][sl, :], hnode[:])
                nc.gpsimd.collective_compute(
                    "AllGather", OP.bypass, replica_groups=RG,
                    ins=[ag_in[li].opt()], outs=[hf[li].opt()],
                )

            layer(0, din["x_rep"][:], xownT_sb, Wl[0], Wr[0])
            layer(1, hf[0][:], hT[0], Wl[1], Wr[1])
            layer(2, hf[1][:], hT[1], Wl[2], Wr[2])

    nc.compile()
    return nc


# --------------------------------------------------------------------------
# Entry point
# --------------------------------------------------------------------------

def prepare(inputs):
    """Host preprocessing: returns (program, per-core input maps, pos)."""
    x = np.asarray(inputs["x"], np.float32)
    edge_index = np.asarray(inputs["edge_index"])

    T, t_total, idx, winv, mb, pos = _preprocess(edge_index)
    nc = _build_program(T, t_total)

    xp = np.zeros((TROWS, F), np.float32)
    xp[pos] = x
    blo = np.asarray(inputs["blo"], np.float32)
    blo_pad = np.full(CP, -1e30, np.float32)
    blo_pad[:C] = blo
    blo_mat = np.broadcast_to(blo_pad[None, :], (P, CP)).copy()

    def padw(a):
        out = np.zeros((H, CP), np.float32)
        out[:, :C] = np.asarray(a, np.float32)
        return out

    ident = np.eye(P, dtype=np.float32)

    def col(v):
        return np.asarray(v, np.float32).reshape(-1, 1)

    in_maps = []
    for c in range(NCORES):
        im = {
            "x_rep": xp,
            "xownT": np.ascontiguousarray(xp[c * RPC : (c + 1) * RPC].T),
            "idx": idx[c],
            "winv": winv[c],
            "ident": ident,
            "identf": ident,
            "mb": mb[c],
            "Wl0": np.asarray(inputs["Wl0"], np.float32),
            "Wr0": np.asarray(inputs["Wr0"], np.float32),
            "bl0": col(inputs["bl0"]),
            "g0": col(inputs["g0"]),
            "b0": col(inputs["b0"]),
            "Wl1": np.asarray(inputs["Wl1"], np.float32),
            "Wr1": np.asarray(inputs["Wr1"], np.float32),
            "bl1": col(inputs["bl1"]),
            "g1": col(inputs["g1"]),
            "b1": col(inputs["b1"]),
            "Wlo": padw(inputs["Wlo"]),
            "Wro": padw(inputs["Wro"]),
            "blo_mat": blo_mat,
        }
        in_maps.append(im)
    return nc, in_maps, pos


def kernel(**inputs):
    global LAST_RESULT
    nc, in_maps, pos = prepare(inputs)
    res = bass_utils.run_bass_kernel_spmd(
        nc, in_maps, core_ids=list(range(NCORES))
    )
    LAST_RESULT = res

    full = np.concatenate(
        [res.results[c]["out_shard"] for c in range(NCORES)], axis=0
    )
    return np.ascontiguousarray(full[pos]).astype(np.float32)


# revision 31
# speedup vs baseline: 1.4179x; 1.4179x over previous
"""Trainium2 Bass kernel for a 3-layer GraphSAGE GNN (EnhancedSAGE).

Reference computation (see problem statement):
    h  = relu(BN(sage_conv(x, A, Wl0, bl0, Wr0), g0, b0))
    h  = relu(BN(sage_conv(h, A, Wl1, bl1, Wr1), g1, b1))
    out = log_softmax(sage_conv(h, A, Wlo, blo, Wro))
with sage_conv(x) = (mean over in-neighbors of x_src) @ Wl + bl + x @ Wr and
BN = batchnorm over the node dimension.

Distribution strategy (8 NeuronCores, graph/data parallel):
  * Nodes are padded to 50176 = 8 cores x 49 blocks x 128 lanes and sharded
    contiguously: core r owns node rows [r*6272, (r+1)*6272).
  * Edges are partitioned by destination on the host into per-core
    "superslots" (256 destination nodes = 2 blocks), padded to 128-edge tiles
    with a uniform tile count across cores (one SPMD program on all 8 cores).
  * x / h tables are replicated in HBM; per-edge source rows are fetched with
    large batched dma_gather DMAs (int16 indices -> lo/hi table split).
  * segment-mean is one-hot matmul on the tensor engine per 128-edge tile:
    aggT[f, 256 dst] += Xg[e, f]^T @ M[e, 256], with M built in one DVE op
    (M[e, d] = (lane[e] == d) * 1/deg) and matmuls in float32r at full PE
    rate (moving dim 256).
  * Activations stay feature-major so BatchNorm scale/shift/ReLU fuse into
    one scalar-engine activation per block; BN stats AllReduce [128, 2];
    layer outputs are transposed per block and AllGathered node-major for
    the next layer's gather.
"""

import numpy as np

import concourse.bass as bass
import concourse.bacc as bacc
import concourse.tile as tile
import concourse.mybir as mybir
from concourse import bass_utils

P = 128
NCORES = 8
SLOTS = 49                 # 128-node blocks per core
SS = (SLOTS + 1) // 2      # 256-node superslots per core (last is 128 wide)
N, E, F, H, C = 50000, 600000, 128, 128, 47
CP = 48                    # class dim padded for f32r matmul (N must be even)
RPC = SLOTS * P            # rows per core (6272)
NPAD = NCORES * RPC        # padded node count (50176)
EPS = 1e-5
K_G = 24                   # edge-tile columns per gather DMA chunk
SPLIT = 32768              # dma_gather int16 index limit (table row split)

f32 = mybir.dt.float32
f32r = mybir.dt.float32r
i32 = mybir.dt.int32
i16 = mybir.dt.int16
AF = mybir.ActivationFunctionType
OP = mybir.AluOpType
AX = mybir.AxisListType
RG = [list(range(NCORES))]

LAST_RESULT = None  # test harness peeks at this for profiling info


def _ss_width(ss):
    return 256 if 2 * ss + 1 < SLOTS else 128


# --------------------------------------------------------------------------
# Host-side preprocessing
# --------------------------------------------------------------------------

def _preprocess(edge_index):
    src = np.asarray(edge_index[0], np.int64)
    dst = np.asarray(edge_index[1], np.int64)
    cnt = np.bincount(dst, minlength=N).astype(np.float32)
    wnode = (1.0 / np.maximum(cnt, 1.0)).astype(np.float32)

    # superslot id per edge: core * SS + (local block pair)
    blk = dst // P
    core = blk // SLOTS
    ssl = (blk - core * SLOTS) // 2
    sid = core * SS + ssl
    NSB = NCORES * SS

    order = np.argsort(sid, kind="stable")
    src_s = src[order]
    dst_s = dst[order]
    sid_s = sid[order]
    is_hi = src_s >= SPLIT

    bc = np.bincount(sid_s, minlength=NSB)
    bc_lo = np.bincount(sid_s[~is_hi], minlength=NSB)
    bc_hi = bc - bc_lo

    TL = (-(-bc_lo.reshape(NCORES, SS) // P)).max(axis=0).astype(np.int64)
    TH = (-(-bc_hi.reshape(NCORES, SS) // P)).max(axis=0).astype(np.int64)
    TL = np.maximum(TL, (TL + TH) == 0)    # each superslot needs >= 1 tile
    tl_total = int(TL.sum())
    th_total = int(TH.sum())
    t_total = tl_total + th_total
    loff = np.zeros(SS + 1, np.int64)
    np.cumsum(TL, out=loff[1:])
    hoff = np.zeros(SS + 1, np.int64)
    np.cumsum(TH, out=hoff[1:])

    bstart = np.zeros(NSB + 1, np.int64)
    np.cumsum(bc, out=bstart[1:])

    # unified tile-column order: all lo tiles (ss-major), then all hi tiles
    lane = np.full((NCORES, P, t_total), 256.0, np.float32)
    w = np.zeros((NCORES, P, t_total), np.float32)
    idxw_lo = np.zeros((NCORES, P, tl_total * 8), np.int16)
    idxw_hi = np.zeros((NCORES, P, max(th_total, 1) * 8), np.int16)

    def fill(c, cap, ucol0, icol0, esrc, elane, ew, idxw, ibase):
        ne = len(esrc)
        pe_src = np.zeros(cap, np.int64)
        pe_src[:ne] = esrc - ibase
        pe_lane = np.full(cap, 256.0, np.float32)
        pe_lane[:ne] = elane
        pe_w = np.zeros(cap, np.float32)
        pe_w[:ne] = ew
        nt = cap // P
        lane[c, :, ucol0 : ucol0 + nt] = pe_lane.reshape(nt, P).T
        w[c, :, ucol0 : ucol0 + nt] = pe_w.reshape(nt, P).T
        wrapped = pe_src.reshape(-1, 16).T.astype(np.int16)  # [16, cap//16]
        idxw[c, :, icol0 * 8 : icol0 * 8 + cap // 16] = np.tile(wrapped, (8, 1))

    for c in range(NCORES):
        for s in range(SS):
            b = c * SS + s
            e0, e1 = bstart[b], bstart[b + 1]
            es = src_s[e0:e1]
            base = (c * SLOTS + 2 * s) * P
            el = (dst_s[e0:e1] - base).astype(np.float32)
            ew = wnode[dst_s[e0:e1]]
            hi = es >= SPLIT
            if TL[s]:
                fill(c, int(TL[s]) * P, int(loff[s]), int(loff[s]),
                     es[~hi], el[~hi], ew[~hi], idxw_lo, 0)
            if TH[s]:
                fill(c, int(TH[s]) * P, tl_total + int(hoff[s]), int(hoff[s]),
                     es[hi], el[hi], ew[hi], idxw_hi, SPLIT)

    # masks zeroing padded node columns; only the last two superslots can
    # contain node ids >= N
    ma = np.zeros((NCORES, P, 256), np.float32)
    mb = np.zeros((NCORES, P, 256), np.float32)
    for c in range(NCORES):
        for s, m in ((SS - 2, ma), (SS - 1, mb)):
            base = (c * SLOTS + 2 * s) * P
            valid = (np.arange(256) + base) < N
            valid &= np.arange(256) < _ss_width(s)
            m[c][:, :] = valid[None, :].astype(np.float32)
    return TL, TH, tl_total, th_total, idxw_lo, idxw_hi, lane, w, ma, mb


# --------------------------------------------------------------------------
# Device program
# --------------------------------------------------------------------------

def _build_program(TL, TH, tl_total, th_total):
    t_total = tl_total + th_total
    nc = bacc.Bacc(
        "TRN2", target_bir_lowering=False, debug=False, num_devices=NCORES
    )

    din = {}
    for name, shape, dt in [
        ("x_rep", [NPAD, F], f32r),
        ("xownT", [P, RPC], f32r),
        ("idxw_lo", [P, tl_total * 8], i16),
        ("idxw_hi", [P, max(th_total, 1) * 8], i16),
        ("lane", [P, t_total], f32),
        ("nlane", [P, t_total], f32),
        ("w", [P, t_total], f32),
        ("nw", [P, t_total], f32),
        ("iota", [P, 256], f32),
        ("ident", [P, P], f32r),
        ("ma", [P, 256], f32),
        ("mb", [P, 256], f32),
        ("Wl0", [F, H], f32r), ("Wr0", [F, H], f32r), ("bl0", [H, 1], f32),
        ("g0", [H, 1], f32), ("b0", [H, 1], f32),
        ("Wl1", [H, H], f32r), ("Wr1", [H, H], f32r), ("bl1", [H, 1], f32),
        ("g1", [H, 1], f32), ("b1", [H, 1], f32),
        ("Wlo", [H, CP], f32r), ("Wro", [H, CP], f32r), ("blo_mat", [P, CP], f32),
    ]:
        din[name] = nc.dram_tensor(name, shape, dt, kind="ExternalInput").ap()
    out_d = nc.dram_tensor("out_shard", [RPC, C], f32, kind="ExternalOutput").ap()

    loff = np.zeros(SS + 1, np.int64)
    np.cumsum(TL, out=loff[1:])
    hoff = np.zeros(SS + 1, np.int64)
    np.cumsum(TH, out=hoff[1:])

    with tile.TileContext(nc) as tc:
        with (
            tc.tile_pool(name="const", bufs=1) as const,
            tc.tile_pool(name="gpool", bufs=2) as gpool,
            tc.tile_pool(name="mpool", bufs=4) as mpool,
            tc.tile_pool(name="work", bufs=3) as work,
            tc.tile_pool(name="vec", bufs=1) as vec,
            tc.tile_pool(name="psA", bufs=2, space="PSUM") as psA,
            tc.tile_pool(name="psB", bufs=2, space="PSUM") as psB,
            tc.tile_pool(name="psT", bufs=2, space="PSUM") as psT,
            tc.tile_pool(name="dram", bufs=1, space="DRAM") as dram,
        ):
            # ---- persistent constants -------------------------------------
            def load(name, dt=f32):
                t = const.tile(list(din[name].shape), dt, name=name + "_sb")
                nc.sync.dma_start(t[:], din[name][:])
                return t

            iota_sb = load("iota")
            m_sb = {SS - 2: load("ma"), SS - 1: load("mb")}
            idxw_lo_sb = load("idxw_lo", i16)
            idxw_hi_sb = load("idxw_hi", i16)
            lane_sb = load("lane")
            nlane_sb = load("nlane")
            w_sb = load("w")
            nw_sb = load("nw")
            xownT_sb = load("xownT", f32r)
            Wl = [load("Wl0", f32r), load("Wl1", f32r), load("Wlo", f32r)]
            Wr = [load("Wr0", f32r), load("Wr1", f32r), load("Wro", f32r)]
            bl = [load("bl0"), load("bl1")]
            gam = [load("g0"), load("g1")]
            bet = [load("b0"), load("b1")]
            blo_mat_sb = load("blo_mat")
            ident = load("ident", f32r)

            hpre = const.tile([P, RPC], f32, name="hpre")
            hT = [
                const.tile([P, RPC], f32r, name="hT0"),
                const.tile([P, RPC], f32r, name="hT1"),
            ]

            hf = [
                dram.tile([NPAD, F], f32r, name="hf0", addr_space="Shared"),
                dram.tile([NPAD, F], f32r, name="hf1", addr_space="Shared"),
            ]
            ag_in = [
                dram.tile([RPC, F], f32r, name="ag_in0"),
                dram.tile([RPC, F], f32r, name="ag_in1"),
            ]

            # ---- batched gather streams -----------------------------------
            class GStream:
                """Streams edge-source rows from a DRAM table into SBUF in
                K_G-tile chunks via dma_gather (consumed in column order)."""

                def __init__(self, table_ap, idxw_sb, total, tag):
                    self.table_ap = table_ap
                    self.idxw = idxw_sb
                    self.total = total
                    self.tag = tag
                    self.gbuf = None
                    self.base = -1

                def col(self, j):
                    if self.gbuf is None or j >= self.base + K_G:
                        assert self.gbuf is None or j == self.base + K_G
                        cols = min(K_G, self.total - j)
                        gbuf = gpool.tile(
                            [P, K_G, F], f32r, name="gbuf", tag=self.tag
                        )
                        nc.gpsimd.dma_gather(
                            out_ap=gbuf[:, :cols, :],
                            in_ap=self.table_ap,
                            idxs_ap=self.idxw[:, j * 8 : (j + cols) * 8],
                            num_idxs=cols * P,
                            num_idxs_reg=cols * P,
                            elem_size=F,
                            single_packet=False,
                        )
                        self.gbuf = gbuf
                        self.base = j
                    return self.gbuf[:, j - self.base, :]

            # ---- one SAGE layer -------------------------------------------
            def layer(li, table_ap, xown, Wl_sb, Wr_sb):
                is_out = li == 2
                if not is_out:
                    sumc = vec.tile([P, SS], f32, name=f"sumc{li}")
                    ssqc = vec.tile([P, SS], f32, name=f"ssqc{li}")
                glo = GStream(table_ap, idxw_lo_sb, tl_total, "glo")
                ghi = (
                    GStream(table_ap[SPLIT:, :], idxw_hi_sb, th_total, "ghi")
                    if th_total
                    else None
                )

                mctr = [0]

                def build_m(ucol, wd):
                    mctr[0] += 1
                    if mctr[0] % 10 < 3:  # ~30% of one-hot builds on DVE
                        m = mpool.tile([P, 256], f32r, name="m_d", tag="m_d")
                        nc.vector.tensor_scalar(
                            out=m[:, :wd],
                            in0=iota_sb[:, :wd],
                            scalar1=lane_sb[:, ucol : ucol + 1],
                            scalar2=w_sb[:, ucol : ucol + 1],
                            op0=OP.is_equal,
                            op1=OP.mult,
                        )
                        return m
                    m = mpool.tile([P, 256], f32r, name="m")
                    tmp = mpool.tile([P, 256], f32, name="tmp", tag="tmp")
                    nc.scalar.activation(
                        tmp[:, :wd], iota_sb[:, :wd], AF.Abs,
                        bias=nlane_sb[:, ucol : ucol + 1],
                    )
                    nc.scalar.activation(
                        m[:, :wd], tmp[:, :wd], AF.Relu,
                        scale=nw_sb[:, ucol : ucol + 1],
                        bias=w_sb[:, ucol : ucol + 1],
                    )
                    return m

                for s in range(SS):
                    wd = _ss_width(s)
                    nt = int(TL[s]) + int(TH[s])
                    aggp = psA.tile([P, 256], f32, name="aggp")
                    k = 0
                    for t in range(int(TL[s])):
                        m = build_m(int(loff[s]) + t, wd)
                        nc.tensor.matmul(
                            aggp[:, :wd],
                            lhsT=glo.col(int(loff[s]) + t),
                            rhs=m[:, :wd],
                            start=(k == 0),
                            stop=(k == nt - 1),
                        )
                        k += 1
                    for t in range(int(TH[s])):
                        m = build_m(tl_total + int(hoff[s]) + t, wd)
                        nc.tensor.matmul(
                            aggp[:, :wd],
                            lhsT=ghi.col(int(hoff[s]) + t),
                            rhs=m[:, :wd],
                            start=(k == 0),
                            stop=(k == nt - 1),
                        )
                        k += 1
                    agg_sb = work.tile([P, 256], f32r, name="agg_sb")
                    nc.vector.tensor_copy(agg_sb[:, :wd], aggp[:, :wd])
                    base = 2 * s * P
                    if not is_out:
                        hp = psB.tile([P, 256], f32, name="hp")
                        nc.tensor.matmul(
                            hp[:, :wd], lhsT=Wl_sb[:],
                            rhs=agg_sb[:, :wd],
                            start=True, stop=False,
                        )
                        nc.tensor.matmul(
                            hp[:, :wd], lhsT=Wr_sb[:],
                            rhs=xown[:, base : base + wd],
                            start=False, stop=True,
                        )
                        hs = hpre[:, base : base + wd]
                        sq = work.tile([P, 256], f32, name="sq")
                        if s >= SS - 2:
                            nc.scalar.activation(
                                hs, hp[:, :wd], AF.Identity, bias=bl[li][:, :1]
                            )
                            nc.vector.tensor_tensor(
                                out=hs, in0=hs, in1=m_sb[s][:, :wd], op=OP.mult
                            )
                            nc.vector.reduce_sum(
                                sumc[:, s : s + 1], hs, axis=AX.X
                            )
                            nc.scalar.activation(
                                sq[:, :wd], hs, AF.Square,
                                accum_out=ssqc[:, s : s + 1],
                            )
                        else:
                            nc.scalar.activation(
                                hs, hp[:, :wd], AF.Identity, bias=bl[li][:, :1],
                                accum_out=sumc[:, s : s + 1],
                            )
                            nc.scalar.activation(
                                sq[:, :wd], hs, AF.Square,
                                accum_out=ssqc[:, s : s + 1],
                            )
                    else:
                        for d in range(wd // P):
                            sl = slice(base + d * P, base + (d + 1) * P)
                            op_ps = psT.tile([P, CP], f32, name="op_ps")
                            nc.tensor.matmul(
                                op_ps[:], lhsT=agg_sb[:, d * P : (d + 1) * P],
                                rhs=Wl_sb[:], start=True, stop=False,
                            )
                            nc.tensor.matmul(
                                op_ps[:], lhsT=xown[:, sl], rhs=Wr_sb[:],
                                start=False, stop=True,
                            )
                            ob = work.tile([P, CP], f32, name="ob")
                            nc.vector.tensor_tensor(
                                out=ob[:], in0=op_ps[:], in1=blo_mat_sb[:],
                                op=OP.add,
                            )
                            mx = work.tile([P, 1], f32, name="mx")
                            nc.vector.reduce_max(mx[:], ob[:], axis=AX.X)
                            mxn = work.tile([P, 1], f32, name="mxn")
                            nc.vector.tensor_scalar_mul(mxn[:], mx[:], -1.0)
                            ex = work.tile([P, CP], f32, name="ex")
                            se = work.tile([P, 1], f32, name="se")
                            nc.scalar.activation(
                                ex[:], ob[:], AF.Exp, bias=mxn[:, :1],
                                accum_out=se[:],
                            )
                            lse = work.tile([P, 1], f32, name="lse")
                            nc.scalar.activation(lse[:], se[:], AF.Ln)
                            tot = work.tile([P, 1], f32, name="tot")
                            nc.vector.tensor_tensor(
                                out=tot[:], in0=lse[:], in1=mx[:], op=OP.add
                            )
                            res = work.tile([P, CP], f32, name="res")
                            nc.vector.tensor_scalar(
                                out=res[:], in0=ob[:], scalar1=tot[:, :1],
                                scalar2=None, op0=OP.subtract,
                            )
                            nc.sync.dma_start(out_d[sl, :], res[:, :C])

                if is_out:
                    return

                # ---- BN statistics (AllReduce) + scale/shift --------------
                S = vec.tile([P, 1], f32, name=f"S{li}")
                SSq = vec.tile([P, 1], f32, name=f"SSq{li}")
                nc.vector.reduce_sum(S[:], sumc[:], axis=AX.X)
                nc.vector.reduce_sum(SSq[:], ssqc[:], axis=AX.X)
                stat = vec.tile([P, 2], f32, name=f"stat{li}")
                nc.vector.tensor_copy(stat[:, 0:1], S[:])
                nc.vector.tensor_copy(stat[:, 1:2], SSq[:])
                cin = dram.tile([P, 2], f32, name=f"cin{li}")
                cout = dram.tile([P, 2], f32, name=f"cout{li}",
                                 addr_space="Shared")
                nc.sync.dma_start(cin[:], stat[:])
                nc.gpsimd.collective_compute(
                    "AllReduce", OP.add, replica_groups=RG,
                    ins=[cin.opt()], outs=[cout.opt()],
                )
                gst = vec.tile([P, 2], f32, name=f"gst{li}")
                nc.sync.dma_start(gst[:], cout[:])
                mu = vec.tile([P, 1], f32, name=f"mu{li}")
                nc.vector.tensor_scalar_mul(mu[:], gst[:, 0:1], 1.0 / N)
                ex2 = vec.tile([P, 1], f32, name=f"ex2{li}")
                nc.vector.tensor_scalar_mul(ex2[:], gst[:, 1:2], 1.0 / N)
                mu2 = vec.tile([P, 1], f32, name=f"mu2{li}")
                nc.vector.tensor_tensor(out=mu2[:], in0=mu[:], in1=mu[:],
                                        op=OP.mult)
                var = vec.tile([P, 1], f32, name=f"var{li}")
                nc.vector.tensor_tensor(out=var[:], in0=ex2[:], in1=mu2[:],
                                        op=OP.subtract)
                sd = vec.tile([P, 1], f32, name=f"sd{li}")
                epsv = vec.tile([P, 1], f32, name=f"epsv{li}")
                nc.vector.memset(epsv[:], EPS)
                nc.scalar.activation(sd[:], var[:], AF.Sqrt, bias=epsv[:, :1])
                rsd = vec.tile([P, 1], f32, name=f"rsd{li}")
                nc.vector.reciprocal(rsd[:], sd[:])
                scl = vec.tile([P, 1], f32, name=f"scl{li}")
                nc.vector.tensor_tensor(out=scl[:], in0=gam[li][:], in1=rsd[:],
                                        op=OP.mult)
                msc = vec.tile([P, 1], f32, name=f"msc{li}")
                nc.vector.tensor_tensor(out=msc[:], in0=mu[:], in1=scl[:],
                                        op=OP.mult)
                sh = vec.tile([P, 1], f32, name=f"sh{li}")
                nc.vector.tensor_tensor(out=sh[:], in0=bet[li][:], in1=msc[:],
                                        op=OP.subtract)

                # ---- phase B: BN+ReLU, transpose, AllGather ---------------
                for s in range(SLOTS):
                    sl = slice(s * P, (s + 1) * P)
                    nc.scalar.activation(
                        hT[li][:, sl], hpre[:, sl], AF.Relu,
                        bias=sh[:, :1], scale=scl[:, :1],
                    )
                    trp = psT.tile([P, P], f32r, name="trp")
                    nc.tensor.transpose(trp[:], hT[li][:, sl], ident[:])
                    hnode = work.tile([P, P], f32r, name="hnode")
                    nc.vector.tensor_copy(hnode[:], trp[:])
                    nc.sync.dma_start(ag_in[li][sl, :], hnode[:])
                nc.gpsimd.collective_compute(
                    "AllGather", OP.bypass, replica_groups=RG,
                    ins=[ag_in[li].opt()], outs=[hf[li].opt()],
                )

            layer(0, din["x_rep"][:], xownT_sb, Wl[0], Wr[0])
            layer(1, hf[0][:], hT[0], Wl[1], Wr[1])
            layer(2, hf[1][:], hT[1], Wl[2], Wr[2])

    nc.compile()
    return nc


# --------------------------------------------------------------------------
# Entry point
# --------------------------------------------------------------------------

def prepare(inputs):
    """Host preprocessing: returns (program, per-core input maps)."""
    x = np.asarray(inputs["x"], np.float32)
    edge_index = np.asarray(inputs["edge_index"])

    (TL, TH, tl_total, th_total, idxw_lo, idxw_hi, lane, w, ma, mb) = (
        _preprocess(edge_index)
    )
    nlane = -lane
    nw = -w
    nc = _build_program(TL, TH, tl_total, th_total)

    xp = np.zeros((NPAD, F), np.float32)
    xp[:N] = x
    blo = np.asarray(inputs["blo"], np.float32)
    blo_pad = np.full(CP, -1e30, np.float32)
    blo_pad[:C] = blo
    blo_mat = np.broadcast_to(blo_pad[None, :], (P, CP)).copy()

    def padw(a):
        out = np.zeros((H, CP), np.float32)
        out[:, :C] = np.asarray(a, np.float32)
        return out
    iota = np.broadcast_to(
        np.arange(256, dtype=np.float32)[None, :], (P, 256)
    ).copy()
    ident = np.eye(P, dtype=np.float32)

    def col(v):
        return np.asarray(v, np.float32).reshape(-1, 1)

    in_maps = []
    for c in range(NCORES):
        im = {
            "x_rep": xp,
            "xownT": np.ascontiguousarray(xp[c * RPC : (c + 1) * RPC].T),
            "idxw_lo": idxw_lo[c],
            "idxw_hi": idxw_hi[c],
            "lane": lane[c],
            "nlane": nlane[c],
            "w": w[c],
            "nw": nw[c],
            "iota": iota,
            "ident": ident,
            "ma": ma[c],
            "mb": mb[c],
            "Wl0": np.asarray(inputs["Wl0"], np.float32),
            "Wr0": np.asarray(inputs["Wr0"], np.float32),
            "bl0": col(inputs["bl0"]),
            "g0": col(inputs["g0"]),
            "b0": col(inputs["b0"]),
            "Wl1": np.asarray(inputs["Wl1"], np.float32),
            "Wr1": np.asarray(inputs["Wr1"], np.float32),
            "bl1": col(inputs["bl1"]),
            "g1": col(inputs["g1"]),
            "b1": col(inputs["b1"]),
            "Wlo": padw(inputs["Wlo"]),
            "Wro": padw(inputs["Wro"]),
            "blo_mat": blo_mat,
        }
        in_maps.append(im)
    return nc, in_maps


def kernel(**inputs):
    global LAST_RESULT
    nc, in_maps = prepare(inputs)
    res = bass_utils.run_bass_kernel_spmd(
        nc, in_maps, core_ids=list(range(NCORES))
    )
    LAST_RESULT = res

    out = np.concatenate(
        [res.results[c]["out_shard"] for c in range(NCORES)], axis=0
    )
    return np.ascontiguousarray(out[:N]).astype(np.float32)


# revision 32
# speedup vs baseline: 1.5495x; 1.0928x over previous
"""Trainium2 Bass kernel for a 3-layer GraphSAGE GNN (EnhancedSAGE).

Reference computation (see problem statement):
    h  = relu(BN(sage_conv(x, A, Wl0, bl0, Wr0), g0, b0))
    h  = relu(BN(sage_conv(h, A, Wl1, bl1, Wr1), g1, b1))
    out = log_softmax(sage_conv(h, A, Wlo, blo, Wro))
with sage_conv(x) = (mean over in-neighbors of x_src) @ Wl + bl + x @ Wr and
BN = batchnorm over the node dimension.

Distribution strategy (8 NeuronCores, graph/data parallel):
  * Nodes are padded to 50176 = 8 cores x 49 blocks x 128 lanes and sharded
    contiguously: core r owns node rows [r*6272, (r+1)*6272).
  * Edges are partitioned by destination on the host into per-core
    "superslots" (256 destination nodes = 2 blocks), padded to 128-edge tiles
    with a uniform tile count across cores (one SPMD program on all 8 cores).
  * x / h tables are replicated in HBM; per-edge source rows are fetched with
    large batched dma_gather DMAs (int16 indices -> lo/hi table split).
  * segment-mean is one-hot matmul on the tensor engine per 128-edge tile:
    aggT[f, 256 dst] += Xg[e, f]^T @ M[e, 256], with M built in one DVE op
    (M[e, d] = (lane[e] == d) * 1/deg) and matmuls in float32r at full PE
    rate (moving dim 256).
  * Activations stay feature-major so BatchNorm scale/shift/ReLU fuse into
    one scalar-engine activation per block; BN stats AllReduce [128, 2];
    layer outputs are transposed per block and AllGathered node-major for
    the next layer's gather.
"""

import numpy as np

import concourse.bass as bass
import concourse.bacc as bacc
import concourse.tile as tile
import concourse.mybir as mybir
from concourse import bass_utils

P = 128
NCORES = 8
SLOTS = 49                 # 128-node blocks per core
SS = (SLOTS + 1) // 2      # 256-node superslots per core (last is 128 wide)
N, E, F, H, C = 50000, 600000, 128, 128, 47
CP = 48                    # class dim padded for f32r matmul (N must be even)
RPC = SLOTS * P            # rows per core (6272)
NPAD = NCORES * RPC        # padded node count (50176)
EPS = 1e-5
K_G = 24                   # edge-tile columns per gather DMA chunk
SPLIT = 32768              # dma_gather int16 index limit (table row split)

f32 = mybir.dt.float32
f32r = mybir.dt.float32r
i32 = mybir.dt.int32
i16 = mybir.dt.int16
AF = mybir.ActivationFunctionType
OP = mybir.AluOpType
AX = mybir.AxisListType
RG = [list(range(NCORES))]

LAST_RESULT = None  # test harness peeks at this for profiling info


def _ss_width(ss):
    return 256 if 2 * ss + 1 < SLOTS else 128


# --------------------------------------------------------------------------
# Host-side preprocessing
# --------------------------------------------------------------------------

def _preprocess(edge_index):
    src = np.asarray(edge_index[0], np.int64)
    dst = np.asarray(edge_index[1], np.int64)
    cnt = np.bincount(dst, minlength=N).astype(np.float32)
    wnode = (1.0 / np.maximum(cnt, 1.0)).astype(np.float32)

    # superslot id per edge: core * SS + (local block pair)
    blk = dst // P
    core = blk // SLOTS
    ssl = (blk - core * SLOTS) // 2
    sid = core * SS + ssl
    NSB = NCORES * SS

    order = np.argsort(sid, kind="stable")
    src_s = src[order]
    dst_s = dst[order]
    sid_s = sid[order]
    is_hi = src_s >= SPLIT

    bc = np.bincount(sid_s, minlength=NSB)
    bc_lo = np.bincount(sid_s[~is_hi], minlength=NSB)
    bc_hi = bc - bc_lo

    TL = (-(-bc_lo.reshape(NCORES, SS) // P)).max(axis=0).astype(np.int64)
    TH = (-(-bc_hi.reshape(NCORES, SS) // P)).max(axis=0).astype(np.int64)
    TL = np.maximum(TL, (TL + TH) == 0)    # each superslot needs >= 1 tile
    tl_total = int(TL.sum())
    th_total = int(TH.sum())
    t_total = tl_total + th_total
    loff = np.zeros(SS + 1, np.int64)
    np.cumsum(TL, out=loff[1:])
    hoff = np.zeros(SS + 1, np.int64)
    np.cumsum(TH, out=hoff[1:])

    bstart = np.zeros(NSB + 1, np.int64)
    np.cumsum(bc, out=bstart[1:])

    # unified tile-column order: all lo tiles (ss-major), then all hi tiles
    lane = np.full((NCORES, P, t_total), 256.0, np.float32)
    w = np.zeros((NCORES, P, t_total), np.float32)
    idxw_lo = np.zeros((NCORES, P, tl_total * 8), np.int16)
    idxw_hi = np.zeros((NCORES, P, max(th_total, 1) * 8), np.int16)

    def fill(c, cap, ucol0, icol0, esrc, elane, ew, idxw, ibase):
        ne = len(esrc)
        pe_src = np.zeros(cap, np.int64)
        pe_src[:ne] = esrc - ibase
        pe_lane = np.full(cap, 256.0, np.float32)
        pe_lane[:ne] = elane
        pe_w = np.zeros(cap, np.float32)
        pe_w[:ne] = ew
        nt = cap // P
        lane[c, :, ucol0 : ucol0 + nt] = pe_lane.reshape(nt, P).T
        w[c, :, ucol0 : ucol0 + nt] = pe_w.reshape(nt, P).T
        wrapped = pe_src.reshape(-1, 16).T.astype(np.int16)  # [16, cap//16]
        idxw[c, :, icol0 * 8 : icol0 * 8 + cap // 16] = np.tile(wrapped, (8, 1))

    for c in range(NCORES):
        for s in range(SS):
            b = c * SS + s
            e0, e1 = bstart[b], bstart[b + 1]
            es = src_s[e0:e1]
            base = (c * SLOTS + 2 * s) * P
            el = (dst_s[e0:e1] - base).astype(np.float32)
            ew = wnode[dst_s[e0:e1]]
            hi = es >= SPLIT
            if TL[s]:
                fill(c, int(TL[s]) * P, int(loff[s]), int(loff[s]),
                     es[~hi], el[~hi], ew[~hi], idxw_lo, 0)
            if TH[s]:
                fill(c, int(TH[s]) * P, tl_total + int(hoff[s]), int(hoff[s]),
                     es[hi], el[hi], ew[hi], idxw_hi, SPLIT)

    # masks zeroing padded node columns; only the last two superslots can
    # contain node ids >= N
    ma = np.zeros((NCORES, P, 256), np.float32)
    mb = np.zeros((NCORES, P, 256), np.float32)
    for c in range(NCORES):
        for s, m in ((SS - 2, ma), (SS - 1, mb)):
            base = (c * SLOTS + 2 * s) * P
            valid = (np.arange(256) + base) < N
            valid &= np.arange(256) < _ss_width(s)
            m[c][:, :] = valid[None, :].astype(np.float32)
    return TL, TH, tl_total, th_total, idxw_lo, idxw_hi, lane, w, ma, mb


# --------------------------------------------------------------------------
# Device program
# --------------------------------------------------------------------------

def _build_program(TL, TH, tl_total, th_total):
    t_total = tl_total + th_total
    nc = bacc.Bacc(
        "TRN2", target_bir_lowering=False, debug=False, num_devices=NCORES
    )

    din = {}
    for name, shape, dt in [
        ("x_rep", [NPAD, F], f32r),
        ("xownT", [P, RPC], f32r),
        ("idxw_lo", [P, tl_total * 8], i16),
        ("idxw_hi", [P, max(th_total, 1) * 8], i16),
        ("lane", [P, t_total], f32),
        ("nlane", [P, t_total], f32),
        ("w", [P, t_total], f32),
        ("nw", [P, t_total], f32),
        ("iota", [P, 256], f32),
        ("ident", [P, P], f32r),
        ("ma", [P, 256], f32),
        ("mb", [P, 256], f32),
        ("Wl0", [F, H], f32r), ("Wr0", [F, H], f32r), ("bl0", [H, 1], f32),
        ("g0", [H, 1], f32), ("b0", [H, 1], f32),
        ("Wl1", [H, H], f32r), ("Wr1", [H, H], f32r), ("bl1", [H, 1], f32),
        ("g1", [H, 1], f32), ("b1", [H, 1], f32),
        ("Wlo", [H, CP], f32r), ("Wro", [H, CP], f32r), ("blo_mat", [P, CP], f32),
    ]:
        din[name] = nc.dram_tensor(name, shape, dt, kind="ExternalInput").ap()
    out_d = nc.dram_tensor("out_shard", [RPC, C], f32, kind="ExternalOutput").ap()

    loff = np.zeros(SS + 1, np.int64)
    np.cumsum(TL, out=loff[1:])
    hoff = np.zeros(SS + 1, np.int64)
    np.cumsum(TH, out=hoff[1:])

    with tile.TileContext(nc) as tc:
        with (
            tc.tile_pool(name="const", bufs=1) as const,
            tc.tile_pool(name="gpool", bufs=2) as gpool,
            tc.tile_pool(name="mpool", bufs=4) as mpool,
            tc.tile_pool(name="work", bufs=3) as work,
            tc.tile_pool(name="vec", bufs=1) as vec,
            tc.tile_pool(name="psA", bufs=2, space="PSUM") as psA,
            tc.tile_pool(name="psB", bufs=2, space="PSUM") as psB,
            tc.tile_pool(name="psT", bufs=2, space="PSUM") as psT,
            tc.tile_pool(name="dram", bufs=1, space="DRAM") as dram,
        ):
            # ---- persistent constants -------------------------------------
            def load(name, dt=f32):
                t = const.tile(list(din[name].shape), dt, name=name + "_sb")
                nc.sync.dma_start(t[:], din[name][:])
                return t

            iota_sb = load("iota")
            m_sb = {SS - 2: load("ma"), SS - 1: load("mb")}
            idxw_lo_sb = load("idxw_lo", i16)
            idxw_hi_sb = load("idxw_hi", i16)
            lane_sb = load("lane")
            nlane_sb = load("nlane")
            w_sb = load("w")
            nw_sb = load("nw")
            xownT_sb = load("xownT", f32r)
            Wl = [load("Wl0", f32r), load("Wl1", f32r), load("Wlo", f32r)]
            Wr = [load("Wr0", f32r), load("Wr1", f32r), load("Wro", f32r)]
            bl = [load("bl0"), load("bl1")]
            gam = [load("g0"), load("g1")]
            bet = [load("b0"), load("b1")]
            blo_mat_sb = load("blo_mat")
            ident = load("ident", f32r)

            hpre = const.tile([P, RPC], f32, name="hpre")
            hT = [
                const.tile([P, RPC], f32r, name="hT0"),
                const.tile([P, RPC], f32r, name="hT1"),
            ]

            hf = [
                dram.tile([NPAD, F], f32r, name="hf0", addr_space="Shared"),
                dram.tile([NPAD, F], f32r, name="hf1", addr_space="Shared"),
            ]
            ag_in = [
                dram.tile([RPC, F], f32r, name="ag_in0"),
                dram.tile([RPC, F], f32r, name="ag_in1"),
            ]

            # ---- batched gather streams -----------------------------------
            class GStream:
                """Streams edge-source rows from a DRAM table into SBUF in
                K_G-tile chunks via dma_gather (consumed in column order)."""

                def __init__(self, table_ap, idxw_sb, total, tag):
                    self.table_ap = table_ap
                    self.idxw = idxw_sb
                    self.total = total
                    self.tag = tag
                    self.gbuf = None
                    self.base = -1

                def col(self, j):
                    if self.gbuf is None or j >= self.base + K_G:
                        assert self.gbuf is None or j == self.base + K_G
                        cols = min(K_G, self.total - j)
                        gbuf = gpool.tile(
                            [P, K_G, F], f32r, name="gbuf", tag=self.tag
                        )
                        nc.gpsimd.dma_gather(
                            out_ap=gbuf[:, :cols, :],
                            in_ap=self.table_ap,
                            idxs_ap=self.idxw[:, j * 8 : (j + cols) * 8],
                            num_idxs=cols * P,
                            num_idxs_reg=cols * P,
                            elem_size=F,
                            single_packet=False,
                        )
                        self.gbuf = gbuf
                        self.base = j
                    return self.gbuf[:, j - self.base, :]

            # ---- one SAGE layer -------------------------------------------
            def layer(li, table_ap, xown, Wl_sb, Wr_sb):
                is_out = li == 2
                if not is_out:
                    sumc = vec.tile([P, SS], f32, name=f"sumc{li}")
                    ssqc = vec.tile([P, SS], f32, name=f"ssqc{li}")
                glo = GStream(table_ap, idxw_lo_sb, tl_total, "glo")
                ghi = (
                    GStream(table_ap[SPLIT:, :], idxw_hi_sb, th_total, "ghi")
                    if th_total
                    else None
                )

                mctr = [0]

                def build_m(ucol, wd):
                    mctr[0] += 1
                    if False:  # ACT-only one-hot builds (DVE suffers GpSimd SBUF-port contention)
                        m = mpool.tile([P, 256], f32r, name="m_d", tag="m_d")
                        nc.vector.tensor_scalar(
                            out=m[:, :wd],
                            in0=iota_sb[:, :wd],
                            scalar1=lane_sb[:, ucol : ucol + 1],
                            scalar2=w_sb[:, ucol : ucol + 1],
                            op0=OP.is_equal,
                            op1=OP.mult,
                        )
                        return m
                    m = mpool.tile([P, 256], f32r, name="m")
                    tmp = mpool.tile([P, 256], f32, name="tmp", tag="tmp")
                    nc.scalar.activation(
                        tmp[:, :wd], iota_sb[:, :wd], AF.Abs,
                        bias=nlane_sb[:, ucol : ucol + 1],
                    )
                    nc.scalar.activation(
                        m[:, :wd], tmp[:, :wd], AF.Relu,
                        scale=nw_sb[:, ucol : ucol + 1],
                        bias=w_sb[:, ucol : ucol + 1],
                    )
                    return m

                for s in range(SS):
                    wd = _ss_width(s)
                    nt = int(TL[s]) + int(TH[s])
                    aggp = psA.tile([P, 256], f32, name="aggp")
                    k = 0
                    for t in range(int(TL[s])):
                        m = build_m(int(loff[s]) + t, wd)
                        nc.tensor.matmul(
                            aggp[:, :wd],
                            lhsT=glo.col(int(loff[s]) + t),
                            rhs=m[:, :wd],
                            start=(k == 0),
                            stop=(k == nt - 1),
                        )
                        k += 1
                    for t in range(int(TH[s])):
                        m = build_m(tl_total + int(hoff[s]) + t, wd)
                        nc.tensor.matmul(
                            aggp[:, :wd],
                            lhsT=ghi.col(int(hoff[s]) + t),
                            rhs=m[:, :wd],
                            start=(k == 0),
                            stop=(k == nt - 1),
                        )
                        k += 1
                    agg_sb = work.tile([P, 256], f32r, name="agg_sb")
                    nc.vector.tensor_copy(agg_sb[:, :wd], aggp[:, :wd])
                    base = 2 * s * P
                    if not is_out:
                        hp = psB.tile([P, 256], f32, name="hp")
                        nc.tensor.matmul(
                            hp[:, :wd], lhsT=Wl_sb[:],
                            rhs=agg_sb[:, :wd],
                            start=True, stop=False,
                        )
                        nc.tensor.matmul(
                            hp[:, :wd], lhsT=Wr_sb[:],
                            rhs=xown[:, base : base + wd],
                            start=False, stop=True,
                        )
                        hs = hpre[:, base : base + wd]
                        sq = work.tile([P, 256], f32, name="sq")
                        if s >= SS - 2:
                            nc.scalar.activation(
                                hs, hp[:, :wd], AF.Identity, bias=bl[li][:, :1]
                            )
                            nc.vector.tensor_tensor(
                                out=hs, in0=hs, in1=m_sb[s][:, :wd], op=OP.mult
                            )
                            nc.vector.reduce_sum(
                                sumc[:, s : s + 1], hs, axis=AX.X
                            )
                            nc.scalar.activation(
                                sq[:, :wd], hs, AF.Square,
                                accum_out=ssqc[:, s : s + 1],
                            )
                        else:
                            nc.scalar.activation(
                                hs, hp[:, :wd], AF.Identity, bias=bl[li][:, :1],
                                accum_out=sumc[:, s : s + 1],
                            )
                            nc.scalar.activation(
                                sq[:, :wd], hs, AF.Square,
                                accum_out=ssqc[:, s : s + 1],
                            )
                    else:
                        for d in range(wd // P):
                            sl = slice(base + d * P, base + (d + 1) * P)
                            op_ps = psT.tile([P, CP], f32, name="op_ps")
                            nc.tensor.matmul(
                                op_ps[:], lhsT=agg_sb[:, d * P : (d + 1) * P],
                                rhs=Wl_sb[:], start=True, stop=False,
                            )
                            nc.tensor.matmul(
                                op_ps[:], lhsT=xown[:, sl], rhs=Wr_sb[:],
                                start=False, stop=True,
                            )
                            ob = work.tile([P, CP], f32, name="ob")
                            nc.vector.tensor_tensor(
                                out=ob[:], in0=op_ps[:], in1=blo_mat_sb[:],
                                op=OP.add,
                            )
                            mx = work.tile([P, 1], f32, name="mx")
                            nc.vector.reduce_max(mx[:], ob[:], axis=AX.X)
                            mxn = work.tile([P, 1], f32, name="mxn")
                            nc.vector.tensor_scalar_mul(mxn[:], mx[:], -1.0)
                            ex = work.tile([P, CP], f32, name="ex")
                            se = work.tile([P, 1], f32, name="se")
                            nc.scalar.activation(
                                ex[:], ob[:], AF.Exp, bias=mxn[:, :1],
                                accum_out=se[:],
                            )
                            lse = work.tile([P, 1], f32, name="lse")
                            nc.scalar.activation(lse[:], se[:], AF.Ln)
                            tot = work.tile([P, 1], f32, name="tot")
                            nc.vector.tensor_tensor(
                                out=tot[:], in0=lse[:], in1=mx[:], op=OP.add
                            )
                            res = work.tile([P, CP], f32, name="res")
                            nc.vector.tensor_scalar(
                                out=res[:], in0=ob[:], scalar1=tot[:, :1],
                                scalar2=None, op0=OP.subtract,
                            )
                            nc.sync.dma_start(out_d[sl, :], res[:, :C])

                if is_out:
                    return

                # ---- BN statistics (AllReduce) + scale/shift --------------
                S = vec.tile([P, 1], f32, name=f"S{li}")
                SSq = vec.tile([P, 1], f32, name=f"SSq{li}")
                nc.vector.reduce_sum(S[:], sumc[:], axis=AX.X)
                nc.vector.reduce_sum(SSq[:], ssqc[:], axis=AX.X)
                stat = vec.tile([P, 2], f32, name=f"stat{li}")
                nc.vector.tensor_copy(stat[:, 0:1], S[:])
                nc.vector.tensor_copy(stat[:, 1:2], SSq[:])
                cin = dram.tile([P, 2], f32, name=f"cin{li}")
                cout = dram.tile([P, 2], f32, name=f"cout{li}",
                                 addr_space="Shared")
                nc.sync.dma_start(cin[:], stat[:])
                nc.gpsimd.collective_compute(
                    "AllReduce", OP.add, replica_groups=RG,
                    ins=[cin.opt()], outs=[cout.opt()],
                )
                gst = vec.tile([P, 2], f32, name=f"gst{li}")
                nc.sync.dma_start(gst[:], cout[:])
                mu = vec.tile([P, 1], f32, name=f"mu{li}")
                nc.vector.tensor_scalar_mul(mu[:], gst[:, 0:1], 1.0 / N)
                ex2 = vec.tile([P, 1], f32, name=f"ex2{li}")
                nc.vector.tensor_scalar_mul(ex2[:], gst[:, 1:2], 1.0 / N)
                mu2 = vec.tile([P, 1], f32, name=f"mu2{li}")
                nc.vector.tensor_tensor(out=mu2[:], in0=mu[:], in1=mu[:],
                                        op=OP.mult)
                var = vec.tile([P, 1], f32, name=f"var{li}")
                nc.vector.tensor_tensor(out=var[:], in0=ex2[:], in1=mu2[:],
                                        op=OP.subtract)
                sd = vec.tile([P, 1], f32, name=f"sd{li}")
                epsv = vec.tile([P, 1], f32, name=f"epsv{li}")
                nc.vector.memset(epsv[:], EPS)
                nc.scalar.activation(sd[:], var[:], AF.Sqrt, bias=epsv[:, :1])
                rsd = vec.tile([P, 1], f32, name=f"rsd{li}")
                nc.vector.reciprocal(rsd[:], sd[:])
                scl = vec.tile([P, 1], f32, name=f"scl{li}")
                nc.vector.tensor_tensor(out=scl[:], in0=gam[li][:], in1=rsd[:],
                                        op=OP.mult)
                msc = vec.tile([P, 1], f32, name=f"msc{li}")
                nc.vector.tensor_tensor(out=msc[:], in0=mu[:], in1=scl[:],
                                        op=OP.mult)
                sh = vec.tile([P, 1], f32, name=f"sh{li}")
                nc.vector.tensor_tensor(out=sh[:], in0=bet[li][:], in1=msc[:],
                                        op=OP.subtract)

                # ---- phase B: BN+ReLU, transpose, AllGather ---------------
                for s in range(SLOTS):
                    sl = slice(s * P, (s + 1) * P)
                    nc.scalar.activation(
                        hT[li][:, sl], hpre[:, sl], AF.Relu,
                        bias=sh[:, :1], scale=scl[:, :1],
                    )
                    trp = psT.tile([P, P], f32r, name="trp")
                    nc.tensor.transpose(trp[:], hT[li][:, sl], ident[:])
                    hnode = work.tile([P, P], f32r, name="hnode")
                    nc.vector.tensor_copy(hnode[:], trp[:])
                    nc.sync.dma_start(ag_in[li][sl, :], hnode[:])
                nc.gpsimd.collective_compute(
                    "AllGather", OP.bypass, replica_groups=RG,
                    ins=[ag_in[li].opt()], outs=[hf[li].opt()],
                )

            layer(0, din["x_rep"][:], xownT_sb, Wl[0], Wr[0])
            layer(1, hf[0][:], hT[0], Wl[1], Wr[1])
            layer(2, hf[1][:], hT[1], Wl[2], Wr[2])

    nc.compile()
    return nc


# --------------------------------------------------------------------------
# Entry point
# --------------------------------------------------------------------------

def prepare(inputs):
    """Host preprocessing: returns (program, per-core input maps)."""
    x = np.asarray(inputs["x"], np.float32)
    edge_index = np.asarray(inputs["edge_index"])

    (TL, TH, tl_total, th_total, idxw_lo, idxw_hi, lane, w, ma, mb) = (
        _preprocess(edge_index)
    )
    nlane = -lane
    nw = -w
    nc = _build_program(TL, TH, tl_total, th_total)

    xp = np.zeros((NPAD, F), np.float32)
    xp[:N] = x
    blo = np.asarray(inputs["blo"], np.float32)
    blo_pad = np.full(CP, -1e30, np.float32)
    blo_pad[:C] = blo
    blo_mat = np.broadcast_to(blo_pad[None, :], (P, CP)).copy()

    def padw(a):
        out = np.zeros((H, CP), np.float32)
        out[:, :C] = np.asarray(a, np.float32)
        return out
    iota = np.broadcast_to(
        np.arange(256, dtype=np.float32)[None, :], (P, 256)
    ).copy()
    ident = np.eye(P, dtype=np.float32)

    def col(v):
        return np.asarray(v, np.float32).reshape(-1, 1)

    in_maps = []
    for c in range(NCORES):
        im = {
            "x_rep": xp,
            "xownT": np.ascontiguousarray(xp[c * RPC : (c + 1) * RPC].T),
            "idxw_lo": idxw_lo[c],
            "idxw_hi": idxw_hi[c],
            "lane": lane[c],
            "nlane": nlane[c],
            "w": w[c],
            "nw": nw[c],
            "iota": iota,
            "ident": ident,
            "ma": ma[c],
            "mb": mb[c],
            "Wl0": np.asarray(inputs["Wl0"], np.float32),
            "Wr0": np.asarray(inputs["Wr0"], np.float32),
            "bl0": col(inputs["bl0"]),
            "g0": col(inputs["g0"]),
            "b0": col(inputs["b0"]),
            "Wl1": np.asarray(inputs["Wl1"], np.float32),
            "Wr1": np.asarray(inputs["Wr1"], np.float32),
            "bl1": col(inputs["bl1"]),
            "g1": col(inputs["g1"]),
            "b1": col(inputs["b1"]),
            "Wlo": padw(inputs["Wlo"]),
            "Wro": padw(inputs["Wro"]),
            "blo_mat": blo_mat,
        }
        in_maps.append(im)
    return nc, in_maps


def kernel(**inputs):
    global LAST_RESULT
    nc, in_maps = prepare(inputs)
    res = bass_utils.run_bass_kernel_spmd(
        nc, in_maps, core_ids=list(range(NCORES))
    )
    LAST_RESULT = res

    out = np.concatenate(
        [res.results[c]["out_shard"] for c in range(NCORES)], axis=0
    )
    return np.ascontiguousarray(out[:N]).astype(np.float32)


# revision 33
# speedup vs baseline: 1.5680x; 1.0119x over previous
"""Trainium2 Bass kernel for a 3-layer GraphSAGE GNN (EnhancedSAGE).

Reference computation (see problem statement):
    h  = relu(BN(sage_conv(x, A, Wl0, bl0, Wr0), g0, b0))
    h  = relu(BN(sage_conv(h, A, Wl1, bl1, Wr1), g1, b1))
    out = log_softmax(sage_conv(h, A, Wlo, blo, Wro))
with sage_conv(x) = (mean over in-neighbors of x_src) @ Wl + bl + x @ Wr and
BN = batchnorm over the node dimension.

Distribution strategy (8 NeuronCores, graph/data parallel):
  * Nodes are padded to 50176 = 8 cores x 49 blocks x 128 lanes and sharded
    contiguously: core r owns node rows [r*6272, (r+1)*6272).
  * Edges are partitioned by destination on the host into per-core
    "superslots" (256 destination nodes = 2 blocks), padded to 128-edge tiles
    with a uniform tile count across cores (one SPMD program on all 8 cores).
  * x / h tables are replicated in HBM; per-edge source rows are fetched with
    large batched dma_gather DMAs (int16 indices -> lo/hi table split).
  * segment-mean is one-hot matmul on the tensor engine per 128-edge tile:
    aggT[f, 256 dst] += Xg[e, f]^T @ M[e, 256], with M built in one DVE op
    (M[e, d] = (lane[e] == d) * 1/deg) and matmuls in float32r at full PE
    rate (moving dim 256).
  * Activations stay feature-major so BatchNorm scale/shift/ReLU fuse into
    one scalar-engine activation per block; BN stats AllReduce [128, 2];
    layer outputs are transposed per block and AllGathered node-major for
    the next layer's gather.
"""

import numpy as np

import concourse.bass as bass
import concourse.bacc as bacc
import concourse.tile as tile
import concourse.mybir as mybir
from concourse import bass_utils

P = 128
NCORES = 8
SLOTS = 49                 # 128-node blocks per core
SS = (SLOTS + 1) // 2      # 256-node superslots per core (last is 128 wide)
N, E, F, H, C = 50000, 600000, 128, 128, 47
CP = 48                    # class dim padded for f32r matmul (N must be even)
RPC = SLOTS * P            # rows per core (6272)
NPAD = NCORES * RPC        # padded node count (50176)
EPS = 1e-5
K_G = 24                   # edge-tile columns per gather DMA chunk
SPLIT = 32768              # dma_gather int16 index limit (table row split)

f32 = mybir.dt.float32
f32r = mybir.dt.float32r
bf16 = mybir.dt.bfloat16
i32 = mybir.dt.int32
i16 = mybir.dt.int16
AF = mybir.ActivationFunctionType
OP = mybir.AluOpType
AX = mybir.AxisListType
RG = [list(range(NCORES))]

LAST_RESULT = None  # test harness peeks at this for profiling info


def _ss_width(ss):
    return 256 if 2 * ss + 1 < SLOTS else 128


# --------------------------------------------------------------------------
# Host-side preprocessing
# --------------------------------------------------------------------------

def _preprocess(edge_index):
    src = np.asarray(edge_index[0], np.int64)
    dst = np.asarray(edge_index[1], np.int64)
    cnt = np.bincount(dst, minlength=N).astype(np.float32)
    wnode = (1.0 / np.maximum(cnt, 1.0)).astype(np.float32)

    # superslot id per edge: core * SS + (local block pair)
    blk = dst // P
    core = blk // SLOTS
    ssl = (blk - core * SLOTS) // 2
    sid = core * SS + ssl
    NSB = NCORES * SS

    order = np.argsort(sid, kind="stable")
    src_s = src[order]
    dst_s = dst[order]
    sid_s = sid[order]
    is_hi = src_s >= SPLIT

    bc = np.bincount(sid_s, minlength=NSB)
    bc_lo = np.bincount(sid_s[~is_hi], minlength=NSB)
    bc_hi = bc - bc_lo

    TL = (-(-bc_lo.reshape(NCORES, SS) // P)).max(axis=0).astype(np.int64)
    TH = (-(-bc_hi.reshape(NCORES, SS) // P)).max(axis=0).astype(np.int64)
    TL = np.maximum(TL, (TL + TH) == 0)    # each superslot needs >= 1 tile
    tl_total = int(TL.sum())
    th_total = int(TH.sum())
    t_total = tl_total + th_total
    loff = np.zeros(SS + 1, np.int64)
    np.cumsum(TL, out=loff[1:])
    hoff = np.zeros(SS + 1, np.int64)
    np.cumsum(TH, out=hoff[1:])

    bstart = np.zeros(NSB + 1, np.int64)
    np.cumsum(bc, out=bstart[1:])

    # unified tile-column order: all lo tiles (ss-major), then all hi tiles
    lane = np.full((NCORES, P, t_total), 256.0, np.float32)
    w = np.zeros((NCORES, P, t_total), np.float32)
    idxw_lo = np.zeros((NCORES, P, tl_total * 8), np.int16)
    idxw_hi = np.zeros((NCORES, P, max(th_total, 1) * 8), np.int16)

    def fill(c, cap, ucol0, icol0, esrc, elane, ew, idxw, ibase):
        ne = len(esrc)
        pe_src = np.zeros(cap, np.int64)
        pe_src[:ne] = esrc - ibase
        pe_lane = np.full(cap, 256.0, np.float32)
        pe_lane[:ne] = elane
        pe_w = np.zeros(cap, np.float32)
        pe_w[:ne] = ew
        nt = cap // P
        lane[c, :, ucol0 : ucol0 + nt] = pe_lane.reshape(nt, P).T
        w[c, :, ucol0 : ucol0 + nt] = pe_w.reshape(nt, P).T
        wrapped = pe_src.reshape(-1, 16).T.astype(np.int16)  # [16, cap//16]
        idxw[c, :, icol0 * 8 : icol0 * 8 + cap // 16] = np.tile(wrapped, (8, 1))

    for c in range(NCORES):
        for s in range(SS):
            b = c * SS + s
            e0, e1 = bstart[b], bstart[b + 1]
            es = src_s[e0:e1]
            base = (c * SLOTS + 2 * s) * P
            el = (dst_s[e0:e1] - base).astype(np.float32)
            ew = wnode[dst_s[e0:e1]]
            hi = es >= SPLIT
            if TL[s]:
                fill(c, int(TL[s]) * P, int(loff[s]), int(loff[s]),
                     es[~hi], el[~hi], ew[~hi], idxw_lo, 0)
            if TH[s]:
                fill(c, int(TH[s]) * P, tl_total + int(hoff[s]), int(hoff[s]),
                     es[hi], el[hi], ew[hi], idxw_hi, SPLIT)

    # masks zeroing padded node columns; only the last two superslots can
    # contain node ids >= N
    ma = np.zeros((NCORES, P, 256), np.float32)
    mb = np.zeros((NCORES, P, 256), np.float32)
    for c in range(NCORES):
        for s, m in ((SS - 2, ma), (SS - 1, mb)):
            base = (c * SLOTS + 2 * s) * P
            valid = (np.arange(256) + base) < N
            valid &= np.arange(256) < _ss_width(s)
            m[c][:, :] = valid[None, :].astype(np.float32)
    return TL, TH, tl_total, th_total, idxw_lo, idxw_hi, lane, w, ma, mb


# --------------------------------------------------------------------------
# Device program
# --------------------------------------------------------------------------

def _build_program(TL, TH, tl_total, th_total):
    t_total = tl_total + th_total
    nc = bacc.Bacc(
        "TRN2", target_bir_lowering=False, debug=False, num_devices=NCORES
    )

    din = {}
    for name, shape, dt in [
        ("x_rep", [NPAD, F], f32r),
        ("xownT", [P, RPC], f32r),
        ("idxw_lo", [P, tl_total * 8], i16),
        ("idxw_hi", [P, max(th_total, 1) * 8], i16),
        ("lane", [P, t_total], f32),
        ("nlane", [P, t_total], f32),
        ("w", [P, t_total], f32),
        ("nw", [P, t_total], f32),
        ("iota", [P, 256], f32),
        ("iotab", [P, 256], bf16),
        ("ident", [P, P], f32r),
        ("ma", [P, 256], f32),
        ("mb", [P, 256], f32),
        ("Wl0", [F, H], f32r), ("Wr0", [F, H], f32r), ("bl0", [H, 1], f32),
        ("g0", [H, 1], f32), ("b0", [H, 1], f32),
        ("Wl1", [H, H], f32r), ("Wr1", [H, H], f32r), ("bl1", [H, 1], f32),
        ("g1", [H, 1], f32), ("b1", [H, 1], f32),
        ("Wlo", [H, CP], f32r), ("Wro", [H, CP], f32r), ("blo_mat", [P, CP], f32),
    ]:
        din[name] = nc.dram_tensor(name, shape, dt, kind="ExternalInput").ap()
    out_d = nc.dram_tensor("out_shard", [RPC, C], f32, kind="ExternalOutput").ap()

    loff = np.zeros(SS + 1, np.int64)
    np.cumsum(TL, out=loff[1:])
    hoff = np.zeros(SS + 1, np.int64)
    np.cumsum(TH, out=hoff[1:])

    with tile.TileContext(nc) as tc:
        with (
            tc.tile_pool(name="const", bufs=1) as const,
            tc.tile_pool(name="gpool", bufs=2) as gpool,
            tc.tile_pool(name="mpool", bufs=6) as mpool,
            tc.tile_pool(name="work", bufs=3) as work,
            tc.tile_pool(name="vec", bufs=1) as vec,
            tc.tile_pool(name="psA", bufs=2, space="PSUM") as psA,
            tc.tile_pool(name="psB", bufs=2, space="PSUM") as psB,
            tc.tile_pool(name="psT", bufs=2, space="PSUM") as psT,
            tc.tile_pool(name="dram", bufs=1, space="DRAM") as dram,
        ):
            # ---- persistent constants -------------------------------------
            def load(name, dt=f32):
                t = const.tile(list(din[name].shape), dt, name=name + "_sb")
                nc.sync.dma_start(t[:], din[name][:])
                return t

            iota_sb = load("iota")
            iotab_sb = load("iotab", bf16)
            m_sb = {SS - 2: load("ma"), SS - 1: load("mb")}
            idxw_lo_sb = load("idxw_lo", i16)
            idxw_hi_sb = load("idxw_hi", i16)
            lane_sb = load("lane")
            nlane_sb = load("nlane")
            w_sb = load("w")
            nw_sb = load("nw")
            xownT_sb = load("xownT", f32r)
            Wl = [load("Wl0", f32r), load("Wl1", f32r), load("Wlo", f32r)]
            Wr = [load("Wr0", f32r), load("Wr1", f32r), load("Wro", f32r)]
            bl = [load("bl0"), load("bl1")]
            gam = [load("g0"), load("g1")]
            bet = [load("b0"), load("b1")]
            blo_mat_sb = load("blo_mat")
            ident = load("ident", f32r)

            hpre = const.tile([P, RPC], f32, name="hpre")
            hT = [
                const.tile([P, RPC], f32r, name="hT0"),
                const.tile([P, RPC], f32r, name="hT1"),
            ]

            hf = [
                dram.tile([NPAD, F], f32r, name="hf0", addr_space="Shared"),
                dram.tile([NPAD, F], f32r, name="hf1", addr_space="Shared"),
            ]
            ag_in = [
                dram.tile([RPC, F], f32r, name="ag_in0"),
                dram.tile([RPC, F], f32r, name="ag_in1"),
            ]

            # ---- batched gather streams -----------------------------------
            class GStream:
                """Streams edge-source rows from a DRAM table into SBUF in
                K_G-tile chunks via dma_gather (consumed in column order)."""

                def __init__(self, table_ap, idxw_sb, total, tag):
                    self.table_ap = table_ap
                    self.idxw = idxw_sb
                    self.total = total
                    self.tag = tag
                    self.gbuf = None
                    self.base = -1

                def col(self, j):
                    if self.gbuf is None or j >= self.base + K_G:
                        assert self.gbuf is None or j == self.base + K_G
                        cols = min(K_G, self.total - j)
                        gbuf = gpool.tile(
                            [P, K_G, F], f32r, name="gbuf", tag=self.tag
                        )
                        nc.gpsimd.dma_gather(
                            out_ap=gbuf[:, :cols, :],
                            in_ap=self.table_ap,
                            idxs_ap=self.idxw[:, j * 8 : (j + cols) * 8],
                            num_idxs=cols * P,
                            num_idxs_reg=cols * P,
                            elem_size=F,
                            single_packet=False,
                        )
                        self.gbuf = gbuf
                        self.base = j
                    return self.gbuf[:, j - self.base, :]

            # ---- one SAGE layer -------------------------------------------
            def layer(li, table_ap, xown, Wl_sb, Wr_sb):
                is_out = li == 2
                if not is_out:
                    sumc = vec.tile([P, SS], f32, name=f"sumc{li}")
                    ssqc = vec.tile([P, SS], f32, name=f"ssqc{li}")
                glo = GStream(table_ap, idxw_lo_sb, tl_total, "glo")
                ghi = (
                    GStream(table_ap[SPLIT:, :], idxw_hi_sb, th_total, "ghi")
                    if th_total
                    else None
                )

                mctr = [0]

                def build_m(ucol, wd):
                    mctr[0] += 1
                    if False:  # ACT-only one-hot builds (DVE suffers GpSimd SBUF-port contention)
                        m = mpool.tile([P, 256], f32r, name="m_d", tag="m_d")
                        nc.vector.tensor_scalar(
                            out=m[:, :wd],
                            in0=iota_sb[:, :wd],
                            scalar1=lane_sb[:, ucol : ucol + 1],
                            scalar2=w_sb[:, ucol : ucol + 1],
                            op0=OP.is_equal,
                            op1=OP.mult,
                        )
                        return m
                    m = mpool.tile([P, 256], f32r, name="m")
                    tmp = mpool.tile([P, 256], bf16, name="tmp", tag="tmp")
                    nc.scalar.activation(
                        tmp[:, :wd], iotab_sb[:, :wd], AF.Abs,
                        bias=nlane_sb[:, ucol : ucol + 1],
                    )
                    nc.scalar.activation(
                        m[:, :wd], tmp[:, :wd], AF.Relu,
                        scale=nw_sb[:, ucol : ucol + 1],
                        bias=w_sb[:, ucol : ucol + 1],
                    )
                    return m

                for s in range(SS):
                    wd = _ss_width(s)
                    nt = int(TL[s]) + int(TH[s])
                    aggp = psA.tile([P, 256], f32, name="aggp")
                    k = 0
                    for t in range(int(TL[s])):
                        m = build_m(int(loff[s]) + t, wd)
                        nc.tensor.matmul(
                            aggp[:, :wd],
                            lhsT=glo.col(int(loff[s]) + t),
                            rhs=m[:, :wd],
                            start=(k == 0),
                            stop=(k == nt - 1),
                        )
                        k += 1
                    for t in range(int(TH[s])):
                        m = build_m(tl_total + int(hoff[s]) + t, wd)
                        nc.tensor.matmul(
                            aggp[:, :wd],
                            lhsT=ghi.col(int(hoff[s]) + t),
                            rhs=m[:, :wd],
                            start=(k == 0),
                            stop=(k == nt - 1),
                        )
                        k += 1
                    agg_sb = work.tile([P, 256], f32r, name="agg_sb")
                    nc.vector.tensor_copy(agg_sb[:, :wd], aggp[:, :wd])
                    base = 2 * s * P
                    if not is_out:
                        hp = psB.tile([P, 256], f32, name="hp")
                        nc.tensor.matmul(
                            hp[:, :wd], lhsT=Wl_sb[:],
                            rhs=agg_sb[:, :wd],
                            start=True, stop=False,
                        )
                        nc.tensor.matmul(
                            hp[:, :wd], lhsT=Wr_sb[:],
                            rhs=xown[:, base : base + wd],
                            start=False, stop=True,
                        )
                        hs = hpre[:, base : base + wd]
                        sq = work.tile([P, 256], f32, name="sq")
                        if s >= SS - 2:
                            nc.scalar.activation(
                                hs, hp[:, :wd], AF.Identity, bias=bl[li][:, :1]
                            )
                            nc.vector.tensor_tensor(
                                out=hs, in0=hs, in1=m_sb[s][:, :wd], op=OP.mult
                            )
                            nc.vector.reduce_sum(
                                sumc[:, s : s + 1], hs, axis=AX.X
                            )
                            nc.scalar.activation(
                                sq[:, :wd], hs, AF.Square,
                                accum_out=ssqc[:, s : s + 1],
                            )
                        else:
                            nc.scalar.activation(
                                hs, hp[:, :wd], AF.Identity, bias=bl[li][:, :1],
                                accum_out=sumc[:, s : s + 1],
                            )
                            nc.scalar.activation(
                                sq[:, :wd], hs, AF.Square,
                                accum_out=ssqc[:, s : s + 1],
                            )
                    else:
                        for d in range(wd // P):
                            sl = slice(base + d * P, base + (d + 1) * P)
                            op_ps = psT.tile([P, CP], f32, name="op_ps")
                            nc.tensor.matmul(
                                op_ps[:], lhsT=agg_sb[:, d * P : (d + 1) * P],
                                rhs=Wl_sb[:], start=True, stop=False,
                            )
                            nc.tensor.matmul(
                                op_ps[:], lhsT=xown[:, sl], rhs=Wr_sb[:],
                                start=False, stop=True,
                            )
                            ob = work.tile([P, CP], f32, name="ob")
                            nc.vector.tensor_tensor(
                                out=ob[:], in0=op_ps[:], in1=blo_mat_sb[:],
                                op=OP.add,
                            )
                            mx = work.tile([P, 1], f32, name="mx")
                            nc.vector.reduce_max(mx[:], ob[:], axis=AX.X)
                            mxn = work.tile([P, 1], f32, name="mxn")
                            nc.vector.tensor_scalar_mul(mxn[:], mx[:], -1.0)
                            ex = work.tile([P, CP], f32, name="ex")
                            se = work.tile([P, 1], f32, name="se")
                            nc.scalar.activation(
                                ex[:], ob[:], AF.Exp, bias=mxn[:, :1],
                                accum_out=se[:],
                            )
                            lse = work.tile([P, 1], f32, name="lse")
                            nc.scalar.activation(lse[:], se[:], AF.Ln)
                            tot = work.tile([P, 1], f32, name="tot")
                            nc.vector.tensor_tensor(
                                out=tot[:], in0=lse[:], in1=mx[:], op=OP.add
                            )
                            res = work.tile([P, CP], f32, name="res")
                            nc.vector.tensor_scalar(
                                out=res[:], in0=ob[:], scalar1=tot[:, :1],
                                scalar2=None, op0=OP.subtract,
                            )
                            nc.sync.dma_start(out_d[sl, :], res[:, :C])

                if is_out:
                    return

                # ---- BN statistics (AllReduce) + scale/shift --------------
                S = vec.tile([P, 1], f32, name=f"S{li}")
                SSq = vec.tile([P, 1], f32, name=f"SSq{li}")
                nc.vector.reduce_sum(S[:], sumc[:], axis=AX.X)
                nc.vector.reduce_sum(SSq[:], ssqc[:], axis=AX.X)
                stat = vec.tile([P, 2], f32, name=f"stat{li}")
                nc.vector.tensor_copy(stat[:, 0:1], S[:])
                nc.vector.tensor_copy(stat[:, 1:2], SSq[:])
                cin = dram.tile([P, 2], f32, name=f"cin{li}")
                cout = dram.tile([P, 2], f32, name=f"cout{li}",
                                 addr_space="Shared")
                nc.sync.dma_start(cin[:], stat[:])
                nc.gpsimd.collective_compute(
                    "AllReduce", OP.add, replica_groups=RG,
                    ins=[cin.opt()], outs=[cout.opt()],
                )
                gst = vec.tile([P, 2], f32, name=f"gst{li}")
                nc.sync.dma_start(gst[:], cout[:])
                mu = vec.tile([P, 1], f32, name=f"mu{li}")
                nc.vector.tensor_scalar_mul(mu[:], gst[:, 0:1], 1.0 / N)
                ex2 = vec.tile([P, 1], f32, name=f"ex2{li}")
                nc.vector.tensor_scalar_mul(ex2[:], gst[:, 1:2], 1.0 / N)
                mu2 = vec.tile([P, 1], f32, name=f"mu2{li}")
                nc.vector.tensor_tensor(out=mu2[:], in0=mu[:], in1=mu[:],
                                        op=OP.mult)
                var = vec.tile([P, 1], f32, name=f"var{li}")
                nc.vector.tensor_tensor(out=var[:], in0=ex2[:], in1=mu2[:],
                                        op=OP.subtract)
                sd = vec.tile([P, 1], f32, name=f"sd{li}")
                epsv = vec.tile([P, 1], f32, name=f"epsv{li}")
                nc.vector.memset(epsv[:], EPS)
                nc.scalar.activation(sd[:], var[:], AF.Sqrt, bias=epsv[:, :1])
                rsd = vec.tile([P, 1], f32, name=f"rsd{li}")
                nc.vector.reciprocal(rsd[:], sd[:])
                scl = vec.tile([P, 1], f32, name=f"scl{li}")
                nc.vector.tensor_tensor(out=scl[:], in0=gam[li][:], in1=rsd[:],
                                        op=OP.mult)
                msc = vec.tile([P, 1], f32, name=f"msc{li}")
                nc.vector.tensor_tensor(out=msc[:], in0=mu[:], in1=scl[:],
                                        op=OP.mult)
                sh = vec.tile([P, 1], f32, name=f"sh{li}")
                nc.vector.tensor_tensor(out=sh[:], in0=bet[li][:], in1=msc[:],
                                        op=OP.subtract)

                # ---- phase B: BN+ReLU, transpose, AllGather ---------------
                for s in range(SLOTS):
                    sl = slice(s * P, (s + 1) * P)
                    nc.scalar.activation(
                        hT[li][:, sl], hpre[:, sl], AF.Relu,
                        bias=sh[:, :1], scale=scl[:, :1],
                    )
                    trp = psT.tile([P, P], f32r, name="trp")
                    nc.tensor.transpose(trp[:], hT[li][:, sl], ident[:])
                    hnode = work.tile([P, P], f32r, name="hnode")
                    nc.vector.tensor_copy(hnode[:], trp[:])
                    nc.sync.dma_start(ag_in[li][sl, :], hnode[:])
                nc.gpsimd.collective_compute(
                    "AllGather", OP.bypass, replica_groups=RG,
                    ins=[ag_in[li].opt()], outs=[hf[li].opt()],
                )

            layer(0, din["x_rep"][:], xownT_sb, Wl[0], Wr[0])
            layer(1, hf[0][:], hT[0], Wl[1], Wr[1])
            layer(2, hf[1][:], hT[1], Wl[2], Wr[2])

    nc.compile()
    return nc


# --------------------------------------------------------------------------
# Entry point
# --------------------------------------------------------------------------

def prepare(inputs):
    """Host preprocessing: returns (program, per-core input maps)."""
    x = np.asarray(inputs["x"], np.float32)
    edge_index = np.asarray(inputs["edge_index"])

    (TL, TH, tl_total, th_total, idxw_lo, idxw_hi, lane, w, ma, mb) = (
        _preprocess(edge_index)
    )
    nlane = -lane
    nw = -w
    nc = _build_program(TL, TH, tl_total, th_total)

    xp = np.zeros((NPAD, F), np.float32)
    xp[:N] = x
    blo = np.asarray(inputs["blo"], np.float32)
    blo_pad = np.full(CP, -1e30, np.float32)
    blo_pad[:C] = blo
    blo_mat = np.broadcast_to(blo_pad[None, :], (P, CP)).copy()

    def padw(a):
        out = np.zeros((H, CP), np.float32)
        out[:, :C] = np.asarray(a, np.float32)
        return out
    iota = np.broadcast_to(
        np.arange(256, dtype=np.float32)[None, :], (P, 256)
    ).copy()
    iotab = iota.astype(mybir.dt.np(bf16))
    ident = np.eye(P, dtype=np.float32)

    def col(v):
        return np.asarray(v, np.float32).reshape(-1, 1)

    in_maps = []
    for c in range(NCORES):
        im = {
            "x_rep": xp,
            "xownT": np.ascontiguousarray(xp[c * RPC : (c + 1) * RPC].T),
            "idxw_lo": idxw_lo[c],
            "idxw_hi": idxw_hi[c],
            "lane": lane[c],
            "nlane": nlane[c],
            "w": w[c],
            "nw": nw[c],
            "iota": iota,
            "iotab": iotab,
            "ident": ident,
            "ma": ma[c],
            "mb": mb[c],
            "Wl0": np.asarray(inputs["Wl0"], np.float32),
            "Wr0": np.asarray(inputs["Wr0"], np.float32),
            "bl0": col(inputs["bl0"]),
            "g0": col(inputs["g0"]),
            "b0": col(inputs["b0"]),
            "Wl1": np.asarray(inputs["Wl1"], np.float32),
            "Wr1": np.asarray(inputs["Wr1"], np.float32),
            "bl1": col(inputs["bl1"]),
            "g1": col(inputs["g1"]),
            "b1": col(inputs["b1"]),
            "Wlo": padw(inputs["Wlo"]),
            "Wro": padw(inputs["Wro"]),
            "blo_mat": blo_mat,
        }
        in_maps.append(im)
    return nc, in_maps


def kernel(**inputs):
    global LAST_RESULT
    nc, in_maps = prepare(inputs)
    res = bass_utils.run_bass_kernel_spmd(
        nc, in_maps, core_ids=list(range(NCORES))
    )
    LAST_RESULT = res

    out = np.concatenate(
        [res.results[c]["out_shard"] for c in range(NCORES)], axis=0
    )
    return np.ascontiguousarray(out[:N]).astype(np.float32)
